# revision 26
# baseline (speedup 1.0000x reference)
"""Multi-head attention Trainium2 kernel (B=4, S=2048, E=1024, H=16).

Sharding: 8 cores = 4 batch groups x 2-way head tensor-parallel.
Core c handles batch b=c//2 and heads [g*8, g*8+8) with g=c%2.
Partial output projections are pair-summed with a chunked bf16
ReduceScatter; each core casts its half back to f32 and writes
interleaved 512-row slabs of batch b's final output.

v3 design notes:
- Single interleaved instruction stream: V projection is a prologue,
  Q/K projections for later head pairs and the output projection are
  woven into the ACT-paced attention loop so the PE fills the slack
  while the activation engine streams exp().
- The scores/exp/P@V stream runs globally across head boundaries:
  P@V trails the scores stream by PV_LAG strips even across heads,
  so the exp stream never stalls at a head switch.
- P@V results spill PSUM->SBUF immediately (raw, with the ones-column
  denominator); normalization (gpsimd partition_broadcast + DVE
  reciprocal/multiply) is deferred off the critical path so collective
  stalls on the gpsimd queue can't block PSUM recycling.
- exp runs on 1024-wide strips; V carries a ones-column so P@V also
  emits the softmax denominator row.
- Output partials go to DRAM in bf16; 4-chunk ReduceScatter overlaps
  the tail; chunks are read back, cast to f32 on DVE, and stored.
"""

import os
import sys

import numpy as np

for _p in ("/opt/trn_rl_repo", "/root/.axon_site/_ro/trn_rl_repo"):
    if os.path.isdir(_p) and _p not in sys.path:
        sys.path.append(_p)

import ml_dtypes  # noqa: E402
from concourse import bacc, mybir, tile  # noqa: E402
from concourse.bass_utils import run_bass_kernel_spmd  # noqa: E402

B, S, E, H, DH = 4, 2048, 1024, 16, 64
N_CORES = 8
TP = 2  # head-parallel factor within a batch
H_LOC = H // TP  # 8 heads per core
EI_LOC = H_LOC * DH  # 512 local rows of the concat dim
N_SB = S // 128  # 16 token blocks
N_EC = E // 128  # 8 contraction chunks
N_QB = S // 512  # 4 query blocks
N_KB = S // 128  # 16 key blocks
N_HP = H_LOC // 2  # 4 head pairs
PV_LAG = 2  # P@V trails the scores/exp stream by this many strips
# fp8 DoublePixel scores (2 moving columns/cycle) vs bf16 zero-padded.
FP8_SCORES = os.environ.get("K_FP8_SCORES", "0") == "1"
# ReduceScatter chunks: (row0, nrows); chunk i fires after out-proj of
# rows [row0, row0+nrows). Small chunks keep the collective stream
# pipelined so the final chunk's latency is the only serial tail.
RS_CHUNKS = [(0, 512), (512, 512), (1024, 1024)]

BF = mybir.dt.bfloat16
F32 = mybir.dt.float32
FP8 = mybir.dt.float8e4
QK_DT = FP8 if FP8_SCORES else BF
EXP = mybir.ActivationFunctionType.Exp
MULT = mybir.AluOpType.mult
DP = mybir.MatmulPerfMode.DoublePixel

_CACHE = {}


def _build():
    nc = bacc.Bacc("TRN2", target_bir_lowering=False, debug=False,
                   num_devices=N_CORES)

    xT_in = nc.declare_dram_parameter("xT", [E, S], BF, isOutput=False)
    wq_in = nc.declare_dram_parameter("wq", [E, EI_LOC], BF, isOutput=False)
    wk_in = nc.declare_dram_parameter("wk", [E, EI_LOC], BF, isOutput=False)
    wv_in = nc.declare_dram_parameter("wv", [E, EI_LOC], BF, isOutput=False)
    woT_in = nc.declare_dram_parameter("woT", [EI_LOC, E], BF, isOutput=False)
    bob_in = nc.declare_dram_parameter("bob", [128, E], F32, isOutput=False)
    y_out = nc.declare_dram_parameter("y", [S // TP, E], F32, isOutput=True)

    y_part = nc.dram_tensor("y_part", [S, E], BF)
    y_chunks = [nc.dram_tensor(f"y_chunk{i}", [n // 2, E], BF)
                for i, (_, n) in enumerate(RS_CHUNKS)]

    inv_sqrt_dh = 1.0 / float(np.sqrt(DH))

    with tile.TileContext(nc) as tc:
        with (
            tc.tile_pool(name="const", bufs=1) as constp,
            tc.tile_pool(name="persist", bufs=1) as persist,
            tc.tile_pool(name="scps", bufs=2, space="PSUM") as scps,
            tc.tile_pool(name="pvps", bufs=2, space="PSUM") as pvps,
            tc.tile_pool(name="mixps", bufs=1, space="PSUM") as mixps,
            tc.tile_pool(name="ptp", bufs=5) as ptp,
            tc.tile_pool(name="pvsb", bufs=6) as pvsb,
            tc.tile_pool(name="smalldn", bufs=2) as smalldn,
            tc.tile_pool(name="denbp", bufs=4) as denbp,
            tc.tile_pool(name="youtp", bufs=2) as youtp,
        ):
            xTp = tc.alloc_tile_pool(name="xTp", bufs=1)
            xT = [xTp.tile([128, S], BF, tag=f"xT{ec}", name=f"xT{ec}")
                  for ec in range(N_EC)]

            # ---- input DMAs spread over the 3 hwdge queues ----
            # Ordered by first use: wv split across all three queues
            # (V-projection prologue starts immediately), then xT in
            # column-major quarter chunks (v_unit(sb) only needs columns
            # <= sb*128), then wq/wk, then woT/bob.
            wv_t = [constp.tile([128, EI_LOC], BF, tag=f"wv{ec}",
                                name=f"wv{ec}") for ec in range(N_EC)]
            queues = (nc.sync, nc.scalar, nc.gpsimd)
            qi = 0
            for ec in range(N_EC):
                for half in range(2):
                    cs = slice(half * 256, (half + 1) * 256)
                    queues[qi % 3].dma_start(wv_t[ec][:, cs],
                                             wv_in[ec * 128:(ec + 1) * 128, cs])
                    qi += 1
            for cb in range(4):
                cs = slice(cb * 512, (cb + 1) * 512)
                for ec in range(N_EC):
                    (nc.sync if ec % 2 == 0 else nc.scalar).dma_start(
                        xT[ec][:, cs], xT_in[ec * 128:(ec + 1) * 128, cs])
            wq_t, wk_t = [], []
            for ec in range(N_EC):
                t_wq = constp.tile([128, EI_LOC], BF, tag=f"wq{ec}",
                                   name=f"wq{ec}")
                nc.sync.dma_start(t_wq[:], wq_in[ec * 128:(ec + 1) * 128, :])
                t_wk = constp.tile([128, EI_LOC], BF, tag=f"wk{ec}",
                                   name=f"wk{ec}")
                nc.scalar.dma_start(t_wk[:], wk_in[ec * 128:(ec + 1) * 128, :])
                wq_t.append(t_wq)
                wk_t.append(t_wk)
            woT_t = []
            for c in range(4):
                t = constp.tile([128, E], BF, tag=f"woT{c}", name=f"woT{c}")
                nc.gpsimd.dma_start(t[:], woT_in[c * 128:(c + 1) * 128, :])
                woT_t.append(t)
            bob = constp.tile([128, E], F32, tag="bob")
            nc.gpsimd.dma_start(bob[:], bob_in[:])

            # ---- persistent SBUF tiles ----
            if FP8_SCORES:
                # fp8 DoublePixel: plain [64, S] operands, no padding.
                QT = [persist.tile([64, S], FP8, tag=f"QT{h}", name=f"QT{h}")
                      for h in range(H_LOC)]
                KT = [persist.tile([64, S], FP8, tag=f"KT{h}", name=f"KT{h}")
                      for h in range(H_LOC)]
            else:
                # bf16: zero-pad rows 64-127 so the scores matmul loads
                # the full PE array (zeros keep the activity monitor
                # happy); pads zeroed on gpsimd, ordered by head.
                QT = [persist.tile([128, S], BF, tag=f"QT{h}", name=f"QT{h}")
                      for h in range(H_LOC)]
                KT = [persist.tile([128, S], BF, tag=f"KT{h}", name=f"KT{h}")
                      for h in range(H_LOC)]
                for h in range(H_LOC):
                    nc.gpsimd.memset(QT[h][64:128, :], 0.0)
                    nc.gpsimd.memset(KT[h][64:128, :], 0.0)
            V = [persist.tile([128, H_LOC, DH + 1], BF, tag=f"V{s}",
                              name=f"V{s}") for s in range(N_SB)]
            # CT split by query half so out-proj for rows 0-1023 has no
            # (tile-granularity) dependency on qp1 normalize writes.
            CT = [[persist.tile([128, S // 2], BF, tag=f"CT{qp}_{c}",
                                name=f"CT{qp}_{c}") for c in range(4)]
                  for qp in range(2)]

            st = dict(rs_fired=0)

            # ---- deferred PE work units (woven into attention slack) ----
            # All big psum tiles are uniformly [128, 1024] per pool tag;
            # projection units only use the first 512 columns.
            def v_unit(sb, pool, tag):
                ps = pool.tile([128, 1024], F32, tag=tag, name="vps")
                for ec in range(N_EC):
                    nc.tensor.matmul(
                        ps[:, 0:EI_LOC], xT[ec][:, sb * 128:(sb + 1) * 128],
                        wv_t[ec][:], start=(ec == 0), stop=(ec == N_EC - 1))
                nc.vector.tensor_copy(V[sb][:, :, 0:DH], ps[:, 0:EI_LOC])
                nc.vector.memset(V[sb][:, :, DH], 1.0)

            def qk_unit(hp, qb, which, pool, tag):
                w = wq_t if which == "q" else wk_t
                dst = QT if which == "q" else KT
                ps = pool.tile([128, 1024], F32, tag=tag, name="qkps")
                for ec in range(N_EC):
                    nc.tensor.matmul(
                        ps[:, 0:512],
                        w[ec][:, hp * 128:(hp + 1) * 128],
                        xT[ec][:, qb * 512:(qb + 1) * 512],
                        start=(ec == 0), stop=(ec == N_EC - 1))
                cols = slice(qb * 512, (qb + 1) * 512)
                for hh in range(2):
                    h = 2 * hp + hh
                    rows = slice(hh * 64, (hh + 1) * 64)
                    nc.vector.tensor_copy(dst[h][0:64, cols], ps[rows, 0:512])

            def outproj_unit(sb, pool, tag):
                ct = CT[sb // 8]
                cs = slice((sb % 8) * 128, (sb % 8 + 1) * 128)
                ys = pool.tile([128, E], F32, tag=tag, name="ys")
                for eo in range(2):
                    for c in range(4):
                        nc.tensor.matmul(
                            ys[:, eo * 512:(eo + 1) * 512],
                            ct[c][:, cs],
                            woT_t[c][:, eo * 512:(eo + 1) * 512],
                            start=(c == 0), stop=(c == 3))
                yt = youtp.tile([128, E], BF, tag="yt", name="yt")
                nc.vector.tensor_add(yt[:], ys[:], bob[:])
                nc.sync.dma_start(y_part[sb * 128:(sb + 1) * 128, :], yt[:])
                # fire any ReduceScatter chunk that just completed (the
                # trigger write is cheap; readback happens at the tail so
                # no queue ever blocks waiting for a collective)
                while st["rs_fired"] < len(RS_CHUNKS):
                    i = st["rs_fired"]
                    r0, n = RS_CHUNKS[i]
                    if (sb + 1) * 128 < r0 + n:
                        break
                    nc.gpsimd.collective_compute(
                        "ReduceScatter", mybir.AluOpType.add,
                        replica_groups=[[0, 1], [2, 3], [4, 5], [6, 7]],
                        ins=[y_part[r0:r0 + n, :]],
                        outs=[y_chunks[i][:]])
                    st["rs_fired"] += 1

            # ---- prologue: V projection + QK projection for pair 0 ----
            # (borrows the scores psum ring, idle until attention starts)
            for sb in range(N_SB):
                v_unit(sb, scps, "sc")
            for qb in range(N_QB):
                qk_unit(0, qb, "q", scps, "sc")
                qk_unit(0, qb, "k", scps, "sc")

            # deferred work queue: QK proj for pairs 1-3 woven during
            # heads 0-5 of qp0; out-proj for rows 0-1023 woven during
            # heads 0-3 of qp1.
            filler = []
            for hp in range(1, N_HP):
                for qb in range(N_QB):
                    filler.append(("qk", hp, qb, "q"))
                    filler.append(("qk", hp, qb, "k"))

            def run_filler():
                if not filler:
                    return
                kind = filler.pop(0)
                if kind[0] == "qk":
                    _, hp, qb, which = kind
                    qk_unit(hp, qb, which, mixps, "mx")
                else:
                    outproj_unit(kind[1], mixps, "mx")

            # ---- attention: one global ACT-paced stream ----
            # Strip g covers (qp, h, kb); P@V for strip g-PV_LAG is
            # issued right after strip g's scores+exp, so the PE always
            # has the next scores ready and exp never waits.
            strips = [(qp, h, kb)
                      for qp in range(N_QB // 2)
                      for h in range(H_LOC)
                      for kb in range(N_KB)]
            n_strips = len(strips)
            pts = {}       # strip idx -> PT tile
            pvt = {}       # (qp, h) -> (pv0, pv1)
            norm_q = []    # deferred normalize ops

            def do_scores(g):
                qp, h, kb = strips[g]
                sp = scps.tile([128, 1024], F32, tag="sc", name="sc")
                for half in range(2):
                    qs = slice((2 * qp + half) * 512,
                               (2 * qp + half + 1) * 512)
                    if FP8_SCORES:
                        nc.tensor.matmul(
                            sp[:, half * 512:(half + 1) * 512],
                            KT[h][:, kb * 128:(kb + 1) * 128],
                            QT[h][:, qs], perf_mode=DP)
                    else:
                        nc.tensor.matmul(
                            sp[:, half * 512:(half + 1) * 512],
                            KT[h][:, kb * 128:(kb + 1) * 128],
                            QT[h][:, qs])
                pt = ptp.tile([128, 1024], BF, tag="pt", name="pt")
                nc.scalar.activation(pt[:], sp[:], EXP, scale=inv_sqrt_dh)
                pts[g] = pt

            def do_pv(g):
                qp, h, kb = strips[g]
                if kb == 0:
                    pvt[(qp, h)] = (
                        pvps.tile([DH + 1, 512], F32, tag="pv", name="pv0"),
                        pvps.tile([DH + 1, 512], F32, tag="pv", name="pv1"))
                pv0, pv1 = pvt[(qp, h)]
                pt = pts.pop(g)
                nc.tensor.matmul(pv0[:], V[kb][:, h, :], pt[:, 0:512],
                                 start=(kb == 0), stop=(kb == N_KB - 1))
                nc.tensor.matmul(pv1[:], V[kb][:, h, :], pt[:, 512:1024],
                                 start=(kb == 0), stop=(kb == N_KB - 1))
                if kb == N_KB - 1:
                    # spill raw P@V to SBUF so the psum pair recycles
                    # immediately; normalize later. The denominator row
                    # moves to partition 0 first (partition_broadcast
                    # replicates partition 0 of its input).
                    for half, pv in ((0, pv0), (1, pv1)):
                        sb_t = pvsb.tile([DH, 512], F32, tag="pvs",
                                         name="pvs")
                        nc.vector.tensor_copy(sb_t[:], pv[0:DH, :])
                        den = smalldn.tile([1, 512], F32, tag="den",
                                           name="den")
                        nc.vector.tensor_copy(den[:], pv[DH:DH + 1, :])
                        denb = denbp.tile([64, 512], F32, tag="denb",
                                          name="denb")
                        nc.gpsimd.partition_broadcast(denb[:], den[:])
                        norm_q.append((qp, h, half, sb_t, denb))

            def flush_norms(keep=0):
                while len(norm_q) > keep:
                    qp, h, half, sb_t, denb = norm_q.pop(0)
                    hp, hh = h // 2, h % 2
                    rows = slice(hh * 64, (hh + 1) * 64)
                    qs = slice(half * 512, (half + 1) * 512)
                    rec = smalldn.tile([64, 512], F32, tag="rec",
                                       name="rec")
                    nc.vector.reciprocal_approx_fast(rec[:], denb[:])
                    nc.vector.tensor_tensor(
                        CT[qp][hp][rows, qs], sb_t[:], rec[:], MULT)

            for g in range(n_strips + PV_LAG):
                if g < n_strips:
                    qp, h, kb = strips[g]
                    if kb == 0:
                        # normalize the head-before-last (its broadcast
                        # has had a full head of slack on gpsimd)
                        flush_norms(keep=2)
                    if (qp, h, kb) == (1, 2, 0):
                        # rows 0-1023 fully normalized -> weave out-proj
                        filler.extend(("op", sb) for sb in range(8))
                    do_scores(g)
                if g >= PV_LAG:
                    do_pv(g - PV_LAG)
                if g < n_strips and strips[g][2] % 4 == 1:
                    run_filler()

            # ---- tail: out-proj for rows 1024-2047 + remaining RS ----
            flush_norms()
            while filler:
                run_filler()
            xTp.release()
            rbp = tc.alloc_tile_pool(name="rbp", bufs=2)

            def readback(i):
                r0, n = RS_CHUNKS[i]
                for blk in range(n // 2 // 128):
                    rb = rbp.tile([128, E], BF, tag="rb", name="rb")
                    nc.sync.dma_start(
                        rb[:], y_chunks[i][blk * 128:(blk + 1) * 128, :])
                    rf = rbp.tile([128, E], F32, tag="rf", name="rf")
                    nc.vector.tensor_copy(rf[:], rb[:])
                    nc.scalar.dma_start(
                        y_out[r0 // 2 + blk * 128:
                              r0 // 2 + (blk + 1) * 128, :], rf[:])

            # chunks 0/1 finished long ago: read them back while the
            # tail out-proj runs; the final chunk drains at the end.
            readback(0)
            readback(1)
            for sb in range(8, N_SB):
                if sb % 2 == 0:
                    outproj_unit(sb, mixps, "mx")
                else:
                    outproj_unit(sb, scps, "sc")
            readback(2)
            rbp.release()

    nc.finalize()
    return nc


def _get_nc():
    if "nc" not in _CACHE:
        _CACHE["nc"] = _build()
    return _CACHE["nc"]


def _make_in_maps(x, wq, wk, wv, wo, bo):
    bf16 = ml_dtypes.bfloat16
    x, wq, wk, wv, wo, bo = (np.asarray(a) for a in (x, wq, wk, wv, wo, bo))
    in_maps = []
    for c in range(N_CORES):
        b, g = c // TP, c % TP
        h0 = g * H_LOC
        xT_l = np.ascontiguousarray(x[b].T).astype(bf16)
        wq_l = np.ascontiguousarray(
            wq[h0:h0 + H_LOC].transpose(1, 0, 2).reshape(E, EI_LOC)).astype(bf16)
        wk_l = np.ascontiguousarray(
            wk[h0:h0 + H_LOC].transpose(1, 0, 2).reshape(E, EI_LOC)).astype(bf16)
        wv_l = np.ascontiguousarray(
            wv[h0:h0 + H_LOC].transpose(1, 0, 2).reshape(E, EI_LOC)).astype(bf16)
        woT_l = np.ascontiguousarray(
            wo[:, g * EI_LOC:(g + 1) * EI_LOC].T).astype(bf16)
        bob = np.broadcast_to(bo.astype(np.float32) / TP, (128, E)).copy()
        in_maps.append({
            "xT": xT_l, "wq": wq_l, "wk": wk_l, "wv": wv_l, "woT": woT_l,
            "bob": bob,
        })
    return in_maps


def _assemble(results):
    out = np.empty((B, S, E), dtype=np.float32)
    for c in range(N_CORES):
        b, g = c // TP, c % TP
        y = results[c]["y"]
        for r0, n in RS_CHUNKS:
            half = n // 2
            out[b, r0 + g * half:r0 + (g + 1) * half, :] = \
                y[r0 // 2:r0 // 2 + half, :]
    return out


def kernel(x, wq, wk, wv, wo, bo):
    nc = _get_nc()
    in_maps = _make_in_maps(x, wq, wk, wv, wo, bo)
    res = run_bass_kernel_spmd(nc, in_maps, list(range(N_CORES)))
    return _assemble(res.results)
